# revision 5
# baseline (speedup 1.0000x reference)
"""BoundaryDiceLoss Trainium2 kernel.

Full inputs: pred (32,5,512,512) f32, target (32,512,512) int. Output: scalar f32 loss.

Strategy: pure data-parallel over batch across 8 NeuronCores (4 images each).
Per image, on-device: softmax over the 5 classes (ACT exp + DVE adds +
exp(-ln s) reciprocal with one Newton step), boundary map from shifted
not-equal compares of the target, and all per-(batch,class) reductions as
fused scalar_tensor_tensor passes with per-partition accumulators, finished
by a gpsimd partition_all_reduce.  Host combines the 8 cores' 6x(4x5) sums
into the dice means (the "all-reduce mean" step) and the final scalar.
"""
import sys

sys.path.insert(0, "/opt/trn_rl_repo")

import numpy as np

NUM_CLASSES = 5
BOUNDARY_WEIGHT = 0.8
EPS = 1e-6
N_CORES = 8

# ---------------------------------------------------------------------------
# device kernel
# ---------------------------------------------------------------------------

_CACHE = {}


def _build(BL, C, H, W):
    """Build + compile the per-core Bacc program.

    BL: local batch (images per core), C: classes, H/W: image size.
    H must be a multiple of 128.  Accumulator layout: 6 family tiles
    (S1,S2,S4,S5,N,M) of [128, BL*C] f32, column = b*C + c; output
    "sums" [6, BL*C] f32 (S1,S2,S4,S5 come out negated).
    """
    import concourse.bacc as bacc
    import concourse.tile as tile
    import concourse.mybir as mybir
    import bass_rust

    AF = mybir.ActivationFunctionType
    OP = mybir.AluOpType
    f32 = mybir.dt.float32
    bf16 = mybir.dt.bfloat16
    i32 = mybir.dt.int32

    S = H // 128          # row-blocks per image
    FW = S * W            # free size of one packed image tile
    HB = W + 2            # hbuf per-block width (zero col, W-1 diffs, zero col, pad)

    nc = bacc.Bacc("TRN2", target_bir_lowering=False, debug=False)
    pred_d = nc.dram_tensor("pred", [BL, C, H, W], f32, kind="ExternalInput").ap()
    targ_d = nc.dram_tensor("target", [BL, H, W], i32, kind="ExternalInput").ap()
    sums_d = nc.dram_tensor("sums", [6, BL * C], f32, kind="ExternalOutput").ap()

    with tile.TileContext(nc) as tc:
        with (
            tc.tile_pool(name="px", bufs=2) as px,
            tc.tile_pool(name="pE", bufs=2) as pE,
            tc.tile_pool(name="ptm", bufs=2) as ptm,
            tc.tile_pool(name="ptu", bufs=1) as ptu,
            tc.tile_pool(name="pf", bufs=1) as pf,
            tc.tile_pool(name="pb", bufs=1) as pb,
            tc.tile_pool(name="pp", bufs=2) as pp,
            tc.tile_pool(name="pacc", bufs=1) as pacc,
        ):
            acc = [
                pacc.tile([128, BL * C], f32, tag=f"acc{i}", name=f"acc{i}")
                for i in range(6)
            ]
            for a in acc:
                nc.vector.memset(a[:], 0.0)
            A_S1, A_S2, A_S4, A_S5, A_N, A_M = acc

            for b in range(BL):
                # ---- target loads: mid, up (row-1), down (row+1), packed
                # [128, S, W]: partition p, block s -> image row s*128+p
                tview = targ_d[b].rearrange("(s p) w -> p s w", p=128)
                t_mid = ptm.tile([128, S, W], i32, tag="tmid")
                nc.sync.dma_start(t_mid[:], tview)

                t_up = ptu.tile([128, S, W], i32, tag="tup")
                # rows s*128+p-1. p>=1: plain shift
                nc.sync.dma_start(t_up[1:128], tview[0:127])
                # p=0, s=0: duplicate row 0 (border => no diff)
                nc.sync.dma_start(t_up[0:1, 0:1], tview[0:1, 0:1])
                if S > 1:
                    # p=0, s>=1: row s*128-1 = row 127 of block s-1
                    nc.sync.dma_start(t_up[0:1, 1:S], tview[127:128, 0 : S - 1])

                t_dn = ptu.tile([128, S, W], i32, tag="tdn")
                # rows s*128+p+1. p<=126: plain shift
                nc.sync.dma_start(t_dn[0:127], tview[1:128])
                if S > 1:
                    # p=127, s<S-1: row (s+1)*128 = row 0 of block s+1
                    nc.sync.dma_start(t_dn[127:128, 0 : S - 1], tview[0:1, 1:S])
                # p=127, s=S-1: duplicate last row
                nc.sync.dma_start(
                    t_dn[127:128, S - 1 : S], tview[127:128, S - 1 : S]
                )

                # ---- int32 -> bf16 casts + t>0 on POOL (1-input ops only)
                tpos = pb.tile([128, FW], bf16, tag="tpos")
                nc.gpsimd.tensor_scalar(
                    tpos[:].rearrange("p (s w) -> p s w", s=S), t_mid[:], 0, None,
                    op0=OP.is_gt,
                )
                t_bf = pb.tile([128, FW], bf16, tag="tbf")
                nc.gpsimd.tensor_copy(
                    t_bf[:].rearrange("p (s w) -> p s w", s=S), t_mid[:]
                )
                tu_bf = pb.tile([128, FW], bf16, tag="tubf")
                nc.gpsimd.tensor_copy(
                    tu_bf[:].rearrange("p (s w) -> p s w", s=S), t_up[:]
                )
                td_bf = pb.tile([128, FW], bf16, tag="tdbf")
                nc.gpsimd.tensor_copy(
                    td_bf[:].rearrange("p (s w) -> p s w", s=S), t_dn[:]
                )

                # ---- neighbor-differs maps on DVE (bf16 compares)
                vn1 = pb.tile([128, FW], bf16, tag="vn1")
                nc.vector.tensor_tensor(vn1[:], tu_bf[:], t_bf[:], op=OP.not_equal)
                vn2 = pb.tile([128, FW], bf16, tag="vn2")
                nc.vector.tensor_tensor(vn2[:], t_bf[:], td_bf[:], op=OP.not_equal)
                tb3 = t_bf[:].rearrange("p (s w) -> p s w", s=S)
                hbuf = pb.tile([128, S, HB], bf16, tag="hbuf")
                nc.gpsimd.memset(hbuf[:, :, 0:1], 0.0)
                nc.gpsimd.memset(hbuf[:, :, W : W + 1], 0.0)
                nc.vector.tensor_tensor(
                    hbuf[:, :, 1:W], tb3[:, :, 0 : W - 1], tb3[:, :, 1:W],
                    op=OP.not_equal,
                )

                # ---- boundary-weight map w on DVE
                anyd = pb.tile([128, FW], bf16, tag="anyd")
                nc.vector.tensor_tensor(anyd[:], vn1[:], vn2[:], op=OP.max)
                nc.vector.tensor_tensor(
                    anyd[:].rearrange("p (s w) -> p s w", s=S),
                    anyd[:].rearrange("p (s w) -> p s w", s=S),
                    hbuf[:, :, 0:W],
                    op=OP.max,
                )
                nc.vector.tensor_tensor(
                    anyd[:].rearrange("p (s w) -> p s w", s=S),
                    anyd[:].rearrange("p (s w) -> p s w", s=S),
                    hbuf[:, :, 1 : W + 1],
                    op=OP.max,
                )
                wmap = pb.tile([128, FW], bf16, tag="wmap")
                nc.vector.tensor_tensor(wmap[:], anyd[:], tpos[:], op=OP.mult)

                # ---- exp(pred) per class
                E = pE.tile([128, C * FW], bf16, tag="E")
                for c in range(C):
                    xc = px.tile([128, FW], f32, tag="xc")
                    nc.sync.dma_start(
                        xc[:].rearrange("p (s w) -> p s w", s=S),
                        pred_d[b, c].rearrange("(s p) w -> p s w", p=128),
                    )
                    nc.scalar.activation(
                        E[:, c * FW : (c + 1) * FW], xc[:], AF.Exp
                    )

                # ---- softmax denominator and -1/s (Newton-refined)
                a01 = pb.tile([128, FW], bf16, tag="a01")
                nc.vector.tensor_tensor(
                    a01[:], E[:, 0:FW], E[:, FW : 2 * FW], op=OP.add
                )
                a23 = pb.tile([128, FW], bf16, tag="a23")
                nc.vector.tensor_tensor(
                    a23[:], E[:, 2 * FW : 3 * FW], E[:, 3 * FW : 4 * FW], op=OP.add
                )
                nc.vector.tensor_tensor(a01[:], a01[:], a23[:], op=OP.add)
                s_f = pf.tile([128, FW], f32, tag="sf")
                nc.vector.tensor_tensor(
                    s_f[:], a01[:], E[:, 4 * FW : 5 * FW], op=OP.add
                )
                ln_s = pf.tile([128, FW], f32, tag="ftmp")
                nc.scalar.activation(ln_s[:], s_f[:], AF.Ln)
                r0 = pf.tile([128, FW], f32, tag="r0")
                nc.scalar.activation(r0[:], ln_s[:], AF.Exp, scale=-1.0)
                # rn = (s*r0 - 2)*r0 = -refined(1/s)
                t1 = pf.tile([128, FW], f32, tag="ftmp")
                nc.vector.scalar_tensor_tensor(
                    t1[:], s_f[:], 1.0, r0[:], op0=OP.mult, op1=OP.mult
                )
                rn = pb.tile([128, FW], bf16, tag="rn")
                nc.vector.scalar_tensor_tensor(
                    rn[:], t1[:], 2.0, r0[:], op0=OP.subtract, op1=OP.mult
                )
                rwn = pb.tile([128, FW], bf16, tag="rwn")
                nc.vector.tensor_tensor(rwn[:], rn[:], wmap[:], op=OP.mult)

                # ---- per-class fused masked sums
                for c in range(C):
                    col = slice(b * C + c, b * C + c + 1)
                    Ec = E[:, c * FW : (c + 1) * FW]
                    Pc = pp.tile([128, FW], bf16, tag="Pc")
                    nc.vector.scalar_tensor_tensor(
                        Pc[:], Ec, 0.0, rn[:], op0=OP.bypass, op1=OP.mult,
                        accum_out=A_S1[:, col],
                    )
                    scr = pp.tile([128, FW], bf16, tag="scr")
                    nc.vector.scalar_tensor_tensor(
                        scr[:], t_bf[:], float(c), Pc[:],
                        op0=OP.is_equal, op1=OP.mult, accum_out=A_S2[:, col],
                    )
                    PWc = pp.tile([128, FW], bf16, tag="PWc")
                    nc.vector.scalar_tensor_tensor(
                        PWc[:], Ec, 0.0, rwn[:], op0=OP.bypass, op1=OP.mult,
                        accum_out=A_S4[:, col],
                    )
                    scr2 = pp.tile([128, FW], bf16, tag="scr")
                    nc.vector.scalar_tensor_tensor(
                        scr2[:], t_bf[:], float(c), PWc[:],
                        op0=OP.is_equal, op1=OP.mult, accum_out=A_S5[:, col],
                    )
                    scr3 = pp.tile([128, FW], bf16, tag="scr")
                    nc.vector.scalar_tensor_tensor(
                        scr3[:], t_bf[:], float(c), t_bf[:],
                        op0=OP.is_equal, op1=OP.bypass, accum_out=A_N[:, col],
                    )
                    scr4 = pp.tile([128, FW], bf16, tag="scr")
                    nc.vector.scalar_tensor_tensor(
                        scr4[:], t_bf[:], float(c), wmap[:],
                        op0=OP.is_equal, op1=OP.mult, accum_out=A_M[:, col],
                    )

            # ---- cross-partition reduce + store
            for i in range(6):
                red = pacc.tile([128, BL * C], f32, tag=f"red{i}", name=f"red{i}")
                nc.gpsimd.partition_all_reduce(
                    red[:], acc[i][:], channels=128,
                    reduce_op=bass_rust.ReduceOp.add,
                )
                nc.sync.dma_start(sums_d[i : i + 1, :], red[0:1, :])

    nc.compile()
    return nc


def _get_nc(BL, C, H, W):
    key = (BL, C, H, W)
    if key not in _CACHE:
        _CACHE[key] = _build(BL, C, H, W)
    return _CACHE[key]


# ---------------------------------------------------------------------------
# host wrapper
# ---------------------------------------------------------------------------


def _finalize(sums_list, BL, C):
    """sums_list: per-core [6, BL*C] arrays -> scalar loss (f64 internally)."""
    A = np.stack([s.reshape(6, BL, C) for s in sums_list]).astype(np.float64)
    A = A.transpose(1, 0, 2, 3).reshape(6, len(sums_list) * BL, C)
    S1, S2, S4, S5, N, M = A
    S1, S2, S4, S5 = -S1, -S2, -S4, -S5
    dice_std = (2.0 * S2 + EPS) / (S1 + N + EPS)
    dice_b = (2.0 * S5 + EPS) / (S4 + M + EPS)
    loss_std = 1.0 - dice_std.mean()
    loss_b = 1.0 - dice_b.mean()
    return np.float32(
        (1.0 - BOUNDARY_WEIGHT) * loss_std + BOUNDARY_WEIGHT * loss_b
    )


def kernel(pred, target):
    from concourse.bass_utils import run_bass_kernel_spmd

    pred = np.ascontiguousarray(np.asarray(pred, dtype=np.float32))
    target = np.ascontiguousarray(np.asarray(target).astype(np.int32))
    B, C, H, W = pred.shape
    assert B % N_CORES == 0
    BL = B // N_CORES

    nc = _get_nc(BL, C, H, W)
    in_maps = [
        {
            "pred": pred[i * BL : (i + 1) * BL],
            "target": target[i * BL : (i + 1) * BL],
        }
        for i in range(N_CORES)
    ]
    res = run_bass_kernel_spmd(nc, in_maps, list(range(N_CORES)))
    return _finalize([res.results[i]["sums"] for i in range(N_CORES)], BL, C)
